# revision 9
# baseline (speedup 1.0000x reference)
"""Windowed dma_gather variant: 4 int16 windows, 4 SWDGE queues, gather
table bounced into a DEDICATED (non-Shared) DRAM pool."""
import numpy as np

import concourse.bacc as bacc
import concourse.bass as bass
import concourse.mybir as mybir
import concourse.tile as tile
from concourse.masks import make_identity

N = 100000
E = 1600000
IN_D, HID, OUT_D = 256, 64, 64
N_CORES = 8
P = 128
W = 4
GSZ = 7
F32 = mybir.dt.float32
I16 = mybir.dt.int16


def _plan(edge_index: np.ndarray, n: int):
    band = N_CORES * P
    n_bands = (n + band - 1) // band
    rpc = n_bands * P
    ntot = N_CORES * rpc
    WR = ntot // W

    src = np.asarray(edge_index[0], dtype=np.int64)
    dst = np.asarray(edge_index[1], dtype=np.int64)
    ne = len(src)
    deg = np.bincount(dst, minlength=n).astype(np.float64) + 1.0

    order = np.argsort(-deg, kind="stable")
    ii = np.arange(ntot)
    gg, ss = ii // band, ii % band
    cc, pp = ss // P, ss % P
    posid = cc * rpc + gg * P + pp
    reserved = np.array([(2 * wi + 1) * rpc + (n_bands - 1) * P + 127
                         for wi in range(W)])
    seq = posid[~np.isin(posid, reserved)]
    new_id = np.empty(n, dtype=np.int64)
    new_id[order] = seq[:n]

    nsrc = new_id[src]
    ndst = new_id[dst]
    win = nsrc // WR

    cnt = np.zeros((ntot, W), np.int32)
    np.add.at(cnt, (ndst, win), 1)
    bnd = (np.arange(ntot) % rpc) // P
    K4 = np.zeros((n_bands, W), np.int64)
    for wi in range(W):
        m = np.zeros(n_bands, np.int64)
        np.maximum.at(m, bnd, cnt[:, wi])
        K4[:, wi] = m

    used = np.zeros(ntot, bool)
    used[new_id] = True
    padrow = np.empty(W, np.int64)
    for wi in range(W):
        free = np.nonzero(~used[wi * WR:(wi + 1) * WR])[0]
        assert len(free) > 0, f"no dummy row in window {wi}"
        padrow[wi] = wi * WR + free[0]

    groups = [list(range(g0, min(g0 + GSZ, n_bands)))
              for g0 in range(0, n_bands, GSZ)]
    suboff = {}
    ccols = {}
    for gi, bands in enumerate(groups):
        for wi in range(W):
            off = 0
            for t in bands:
                suboff[(gi, wi, t)] = off
                off += int(K4[t, wi])
            ccols[(gi, wi)] = off

    eo = np.argsort(ndst, kind="stable")
    sdst, ssrc, swin = ndst[eo], nsrc[eo], win[eo]
    ordw = np.lexsort((np.arange(ne), swin, sdst))
    sdst, ssrc, swin = sdst[ordw], ssrc[ordw], swin[ordw]
    key = sdst * W + swin
    first = np.searchsorted(key, key, side="left")
    kidx = np.arange(ne) - first

    ec = sdst // rpc
    loc = sdst % rpc
    eg = loc // P
    ep = loc % P
    egi = eg // GSZ

    call_list = [(gi, wi) for gi in range(len(groups)) for wi in range(W)]
    call_cols = [ccols[c] for c in call_list]
    call_off = np.concatenate([[0], np.cumsum([c * P for c in call_cols])])
    tot_j = int(call_off[-1])
    SLOTS4 = int(sum(call_cols))

    flat = np.empty((N_CORES, tot_j), np.int16)
    for ci, (gi, wi) in enumerate(call_list):
        base = padrow[wi] - wi * WR
        flat[:, call_off[ci]:call_off[ci + 1]] = np.int16(base)

    call_idx = {c: i for i, c in enumerate(call_list)}
    e_call = np.array([call_idx[(g_, w_)] for g_, w_ in zip(egi, swin)])
    col = (np.array([suboff[(g_, w_, t_)]
                     for g_, w_, t_ in zip(egi, swin, eg)]) + kidx)
    j = call_off[e_call] + col * P + ep
    flat[ec, j] = (ssrc - swin * WR).astype(np.int16)

    wrapped = np.empty((N_CORES, 16, tot_j // 16), np.int16)
    for ci in range(len(call_list)):
        lo, hi = call_off[ci], call_off[ci + 1]
        blk = flat[:, lo:hi].reshape(N_CORES, (hi - lo) // 16, 16)
        wrapped[:, :, lo // 16:hi // 16] = blk.transpose(0, 2, 1)
    sidx16 = np.tile(wrapped, (1, 8, 1))

    dr = (1.0 / np.sqrt(deg)).astype(np.float32)
    di = (1.0 / deg).astype(np.float32)
    drn = np.zeros(ntot, np.float32)
    din = np.zeros(ntot, np.float32)
    drn[new_id] = dr
    din[new_id] = di
    degc = np.zeros((N_CORES, P, 2 * n_bands), np.float32)
    degc[:, :, :n_bands] = drn.reshape(N_CORES, n_bands, P).transpose(0, 2, 1)
    degc[:, :, n_bands:] = din.reshape(N_CORES, n_bands, P).transpose(0, 2, 1)

    return dict(new_id=new_id, K4=K4, groups=groups, suboff=suboff,
                ccols=ccols, call_list=call_list, call_off=call_off,
                sidx16=sidx16, degc=degc, tot_j=tot_j, SLOTS4=SLOTS4,
                n_bands=n_bands, rpc=rpc, ntot=ntot, WR=WR, n=n)


def _build(pl, in_d=IN_D, no_gather=False, no_coll=False, no_reduce=False):
    K4, groups = pl["K4"], pl["groups"]
    suboff, ccols, call_list = pl["suboff"], pl["ccols"], pl["call_list"]
    call_off, tot_j = pl["call_off"], pl["tot_j"]
    n_bands, rpc, ntot, WR = pl["n_bands"], pl["rpc"], pl["ntot"], pl["WR"]

    nc = bacc.Bacc("TRN2", target_bir_lowering=False, debug=False,
                   num_devices=N_CORES, num_swdge_queues=4)
    xT = nc.dram_tensor("xT", [in_d, rpc], F32, kind="ExternalInput")
    W1 = nc.dram_tensor("W1", [in_d, HID], F32, kind="ExternalInput")
    W2 = nc.dram_tensor("W2", [HID, HID], F32, kind="ExternalInput")
    W3 = nc.dram_tensor("W3", [HID, OUT_D], F32, kind="ExternalInput")
    Pw1 = nc.dram_tensor("Pw1", [OUT_D, HID], F32, kind="ExternalInput")
    Pw2 = nc.dram_tensor("Pw2", [HID, OUT_D], F32, kind="ExternalInput")
    brep = nc.dram_tensor("brep", [P, 5 * 64], F32, kind="ExternalInput")
    degc = nc.dram_tensor("degc", [P, 2 * n_bands], F32, kind="ExternalInput")
    sidx = nc.dram_tensor("sidx", [P, tot_j // 16], I16,
                          kind="ExternalInput")
    z = nc.dram_tensor("z", [rpc, 64], F32, kind="ExternalOutput")

    kchunks = (in_d + P - 1) // P
    with tile.TileContext(nc) as tc:
        with (
            tc.tile_pool(name="const", bufs=1) as cpool,
            tc.tile_pool(name="acts", bufs=1) as apool,
            tc.tile_pool(name="xin", bufs=3) as xpool,
            tc.tile_pool(name="work", bufs=3) as wpool,
            tc.tile_pool(name="gbuf", bufs=2) as gpool,
            tc.tile_pool(name="psmm", bufs=4, space="PSUM") as psmm,
            tc.tile_pool(name="pstr", bufs=4, space="PSUM") as pstr,
            tc.tile_pool(name="dram", bufs=2, space="DRAM") as dpool,
            tc.tile_pool(name="ldram", bufs=2, space="DRAM") as lpool,
        ):
            w1s = []
            for kc in range(kchunks):
                kp = min(P, in_d - kc * P)
                wc = cpool.tile([kp, HID], F32, tag=f"w1_{kc}")
                nc.sync.dma_start(wc[:], W1[kc * P:kc * P + kp, :])
                w1s.append(wc)
            w2 = cpool.tile([HID, HID], F32, tag="w2")
            w3 = cpool.tile([HID, OUT_D], F32, tag="w3")
            pw1 = cpool.tile([OUT_D, HID], F32, tag="pw1")
            pw2 = cpool.tile([HID, OUT_D], F32, tag="pw2")
            nc.sync.dma_start(w2[:], W2[:])
            nc.sync.dma_start(w3[:], W3[:])
            nc.sync.dma_start(pw1[:], Pw1[:])
            nc.sync.dma_start(pw2[:], Pw2[:])
            bsb = cpool.tile([P, 5 * 64], F32, tag="bsb")
            dsb = cpool.tile([P, 2 * n_bands], F32, tag="dsb")
            isb = cpool.tile([P, tot_j // 16], I16, tag="isb")
            nc.sync.dma_start(bsb[:], brep[:])
            nc.sync.dma_start(dsb[:], degc[:])
            nc.sync.dma_start(isb[:], sidx[:])
            ident = cpool.tile([P, P], F32, tag="ident")
            make_identity(nc, ident[:])

            p_sb = apool.tile([P, n_bands * 64], F32, tag="p_sb")
            actA = apool.tile([P, n_bands * 64], F32, tag="actA")
            AGRP = 14

            act = None
            qrot = 0
            for L in range(3):
                ag_in = dpool.tile([rpc, 64], F32, tag="ag_in")
                table = dpool.tile([ntot, 64], F32, tag="table",
                                   addr_space="Shared")
                ltab = lpool.tile([ntot, 64], F32, tag="ltab")
                wl = [None, w2, w3][L]
                for t in range(n_bands):
                    tb = slice(t * 64, (t + 1) * 64)
                    ps_h = psmm.tile([P, 64], F32, tag="ps_h")
                    if L == 0:
                        for kc in range(kchunks):
                            kp = min(P, in_d - kc * P)
                            xc = xpool.tile([kp, P], F32, tag=f"xc{kc}")
                            nc.sync.dma_start(
                                xc[:], xT[kc * P:kc * P + kp,
                                          t * P:(t + 1) * P])
                            nc.tensor.matmul(ps_h[:], xc[:], w1s[kc][:],
                                             start=(kc == 0),
                                             stop=(kc == kchunks - 1))
                    else:
                        ps_tr = pstr.tile([64, P], F32, tag="ps_tr")
                        nc.tensor.transpose(ps_tr[:], act[:, tb], ident[:])
                        lh = wpool.tile([64, P], F32, tag="lh")
                        nc.vector.tensor_copy(lh[:], ps_tr[:])
                        nc.tensor.matmul(ps_h[:], lh[:], wl[:],
                                         start=True, stop=True)
                    nc.vector.tensor_scalar_mul(p_sb[:, tb], ps_h[:],
                                                dsb[:, t:t + 1])
                    if t % AGRP == AGRP - 1 or t == n_bands - 1:
                        g0 = (t // AGRP) * AGRP
                        nc.sync.dma_start(
                            ag_in[g0 * P:(t + 1) * P, :].rearrange(
                                "(tt p) f -> p tt f", p=P),
                            p_sb[:, g0 * 64:(t + 1) * 64].rearrange(
                                "p (tt f) -> p tt f", f=64))

                if not no_coll:
                    nc.gpsimd.collective_compute(
                        "AllGather",
                        mybir.AluOpType.bypass,
                        replica_groups=[list(range(N_CORES))],
                        ins=[ag_in.opt()],
                        outs=[table.opt()],
                    )
                # bounce to a clean local pool (Shared-space random reads
                # are ~3-5x slower)
                for sc in range(4):
                    sl = slice(sc * (ntot // 4), (sc + 1) * (ntot // 4))
                    nc.sync.dma_start(ltab[sl], table[sl])

                for gi, bands in enumerate(groups):
                    nb = len(bands)
                    accg = gpool.tile([P, nb * 64], F32, tag="accg")
                    nc.vector.memset(accg[:], 0.0)
                    for wi in range(W):
                        C = ccols[(gi, wi)]
                        if C == 0 or no_gather:
                            continue
                        ci = call_list.index((gi, wi))
                        lo = int(call_off[ci]) // 16
                        hi = int(call_off[ci + 1]) // 16
                        gt = gpool.tile([P, C, 64], F32, tag="g")
                        ni = C * P
                        nc.gpsimd.dma_gather(
                            out_ap=gt[:],
                            in_ap=ltab[wi * WR:(wi + 1) * WR, :],
                            idxs_ap=isb[:, lo:hi],
                            num_idxs=ni,
                            num_idxs_reg=ni,
                            elem_size=64,
                            queue_num=qrot % 4,
                            single_packet=False,
                        )
                        qrot += 1
                        if no_reduce:
                            continue
                        for ti, t in enumerate(bands):
                            Kw = int(K4[t, wi])
                            if Kw == 0:
                                continue
                            so = suboff[(gi, wi, t)]
                            ab = slice(ti * 64, (ti + 1) * 64)
                            part = wpool.tile([P, 64], F32, tag="part")
                            nc.vector.reduce_sum(
                                out=part[:],
                                in_=gt[:, so:so + Kw].rearrange(
                                    "p k f -> p f k"),
                                axis=mybir.AxisListType.X)
                            nc.vector.tensor_add(accg[:, ab], accg[:, ab],
                                                 part[:])
                    for ti, t in enumerate(bands):
                        tb = slice(t * 64, (t + 1) * 64)
                        ab = slice(ti * 64, (ti + 1) * 64)
                        t1 = wpool.tile([P, 64], F32, tag="t1")
                        nc.vector.tensor_add(t1[:], accg[:, ab],
                                             p_sb[:, tb])
                        nc.vector.tensor_scalar_mul(t1[:], t1[:],
                                                    dsb[:, t:t + 1])
                        nc.vector.tensor_add(t1[:], t1[:],
                                             bsb[:, L * 64:(L + 1) * 64])
                        nc.scalar.activation(
                            actA[:, tb], t1[:],
                            mybir.ActivationFunctionType.Relu)
                act = actA

            for t in range(n_bands):
                tb = slice(t * 64, (t + 1) * 64)
                ps_tr = pstr.tile([64, P], F32, tag="ps_tr")
                nc.tensor.transpose(ps_tr[:], act[:, tb], ident[:])
                lh = wpool.tile([64, P], F32, tag="lh")
                nc.vector.tensor_copy(lh[:], ps_tr[:])
                ps_q = psmm.tile([P, 64], F32, tag="ps_h")
                nc.tensor.matmul(ps_q[:], lh[:], pw1[:], start=True, stop=True)
                q0 = wpool.tile([P, 64], F32, tag="q0")
                nc.vector.tensor_add(q0[:], ps_q[:], bsb[:, 3 * 64:4 * 64])
                q = wpool.tile([P, 64], F32, tag="q")
                nc.scalar.activation(q[:], q0[:],
                                     mybir.ActivationFunctionType.Relu)
                ps_tr2 = pstr.tile([64, P], F32, tag="ps_tr")
                nc.tensor.transpose(ps_tr2[:], q[:], ident[:])
                lh2 = wpool.tile([64, P], F32, tag="lh")
                nc.vector.tensor_copy(lh2[:], ps_tr2[:])
                ps_z = psmm.tile([P, 64], F32, tag="ps_h")
                nc.tensor.matmul(ps_z[:], lh2[:], pw2[:], start=True,
                                 stop=True)
                nc.vector.tensor_add(p_sb[:, tb], ps_z[:],
                                     bsb[:, 4 * 64:5 * 64])
                if t % AGRP == AGRP - 1 or t == n_bands - 1:
                    g0 = (t // AGRP) * AGRP
                    nc.sync.dma_start(
                        z[g0 * P:(t + 1) * P, :].rearrange(
                            "(tt p) f -> p tt f", p=P),
                        p_sb[:, g0 * 64:(t + 1) * 64].rearrange(
                            "p (tt f) -> p tt f", f=64))

    nc.compile()
    return nc


def _in_maps(inputs, pl, in_d=IN_D):
    x = np.asarray(inputs["x"], np.float32)
    new_id = pl["new_id"]
    rpc, ntot = pl["rpc"], pl["ntot"]
    xn = np.zeros((ntot, in_d), np.float32)
    xn[new_id] = x
    brep = np.tile(
        np.concatenate([
            np.asarray(inputs["b1"], np.float32),
            np.asarray(inputs["b2"], np.float32),
            np.asarray(inputs["b3"], np.float32),
            np.asarray(inputs["Pb1"], np.float32),
            np.asarray(inputs["Pb2"], np.float32),
        ])[None, :], (P, 1))
    common = dict(
        W1=np.asarray(inputs["W1"], np.float32),
        W2=np.asarray(inputs["W2"], np.float32),
        W3=np.asarray(inputs["W3"], np.float32),
        Pw1=np.asarray(inputs["Pw1"], np.float32),
        Pw2=np.asarray(inputs["Pw2"], np.float32),
        brep=brep,
    )
    maps = []
    for c in range(N_CORES):
        xc = xn[c * rpc:(c + 1) * rpc]
        maps.append(dict(
            xT=np.ascontiguousarray(xc.T),
            degc=pl["degc"][c],
            sidx=pl["sidx16"][c],
            **common,
        ))
    return maps


def build_all(inputs, n=None, in_d=IN_D, **bkw):
    x = np.asarray(inputs["x"])
    n = x.shape[0] if n is None else n
    pl = _plan(np.asarray(inputs["edge_index"]), n)
    nc = _build(pl, in_d=in_d, **bkw)
    maps = _in_maps(inputs, pl, in_d=in_d)
    return nc, maps, pl


def postprocess(results, pl):
    z_new = np.concatenate([results[c]["z"] for c in range(N_CORES)], axis=0)
    return np.ascontiguousarray(z_new[pl["new_id"]]).astype(np.float32)


def kernel(**inputs) -> np.ndarray:
    from concourse.bass_utils import run_bass_kernel_spmd
    nc, maps, pl = build_all(inputs)
    res = run_bass_kernel_spmd(nc, maps, core_ids=list(range(N_CORES)))
    return postprocess(res.results, pl)
